# revision 12
# baseline (speedup 1.0000x reference)
"""Trainium2 kernel for nn_LinearDynamics: chunked two-level scan, 8-core data parallel.

v6: - fp8 DoubleRow phase A (4 matmuls/group, each contracting 2 pair-packed
      u blocks); stationaries scaled by SA=32 to keep e4m3 in normal range,
      unscaled in the ACT escape copy.
    - input DMAs priority-ordered and fine-sliced to stream-match phase A
      consumption (early phase A is input-HBM-bandwidth gated).
    - warmup: 8 N=128 matmuls + 4 N=512 zero-matmuls pre-filling the psC
      banks so phase B's d=0 identity term becomes an ACT copy (saves 4
      N=512 matmuls).
    - tail output DMAs (k=14 slab, k=15/16 singles) split across both HWDGE
      queues and issued after the step's copies, removing the serialized
      sync-queue tail.

Per core (128 batch rows, state transposed xT [d_x=128, b=128]):
  x_{t+1} = x_t + (x_t @ dtA + u_t @ B2),  dtA = dt*A, B2 = dt*B, M = I + dtA

Chunks: S=16 chunks of L=16 steps, grouped NG=4 x GS=4 (GW=512 cols).
Host precomputes (float64, cast bf16):
  Wt slot i (i<8): rows 0:64 = N_{15-2i}, rows 64:128 = N_{14-2i}, N_p = B2@M^p
  Wt slot 8/9: B2 zero-padded to rows 0:64 / rows 64:128; slot 10: dtA
  MP_d = M^(d*L)  d=0..4

Phase A: W_s = sum_j u_{sL+j} @ N_{15-j}; u pair-packed on partitions,
  8 matmuls/group into PSUM; ACT copies psum -> Wext (bf16).
Phase B': boundary X-block(g) into psC[g]: 4 windowed W-terms (+ for g>0
  one chain matmul X-block(g-1) @ M^{4L}). Group accumulation left OPEN.
Phase C: PE keeps accumulating into the same psum bank:
    psC[g] += u_j @ B2pad + xr_{k-1} @ dtA     (psum IS the f32 state)
  one copy per step (DVE/ACT alternate) -> bf16 xr tile = next matmul
  input AND DMA output.

Host-sim rel err ~3.8e-3.
"""

import ml_dtypes
import numpy as np

DT = 0.1
BATCH, T, DX, DU = 1024, 256, 128, 64
NCORES = 8
BPC = BATCH // NCORES  # 128
S, L = 16, 16
NG, GS = 4, 4
GW = GS * BPC  # 512
SA = 32.0  # phase A DoubleRow stationary scale (keeps fp8 e4m3 in normal range)

_CACHE = {}


def _build(debug=False):
    import concourse.mybir as mybir
    import concourse.tile as tile
    from concourse import bacc

    f32 = mybir.dt.float32
    bf16 = mybir.dt.bfloat16
    fp8 = mybir.dt.float8e4

    DR = mybir.MatmulPerfMode.DoubleRow

    nc = bacc.Bacc("TRN2", target_bir_lowering=False, debug=debug)
    wdr_d = nc.declare_dram_parameter("WDR", [DX, 8, 2, DX], fp8, isOutput=False)
    wt_d = nc.declare_dram_parameter("WT", [DX, 3 * DX], bf16, isOutput=False)
    mp_d = nc.declare_dram_parameter("MP", [DX, 5 * DX], bf16, isOutput=False)
    w0_d = nc.declare_dram_parameter("W0T", [DX, 4 * DX], bf16, isOutput=False)
    u_d = nc.declare_dram_parameter("uT", [NG, DX, 8 * GW], fp8, isOutput=False)
    y_d = nc.declare_dram_parameter("yT", [NG, 8, DX, 2 * GW], bf16, isOutput=True)

    with tile.TileContext(nc) as tc:
        with (
            tc.tile_pool(name="cw", bufs=1) as cw,
            tc.tile_pool(name="psA", bufs=2, space="PSUM") as psA,
            tc.tile_pool(name="psW", bufs=1, space="PSUM") as psW,
            tc.tile_pool(name="psC", bufs=1, space="PSUM") as psC,
        ):
            # Input DMAs, priority-ordered: early phase A is gated by input
            # HBM bandwidth (~0.28 MB/us), so the pieces phase A consumes
            # first go first, in fine slices that stream-match consumption.
            # Weights on the ACT queue, u on the SYNC queue.
            WDR = cw.tile([DX, 8, 2, DX], fp8)
            Wt = cw.tile([DX, 3 * DX], bf16)
            MP = cw.tile([DX, 5 * DX], bf16)
            Wext = cw.tile([DX, (4 + S) * DX], bf16)
            nc.scalar.dma_start(WDR[:, 0:2, :, :], wdr_d[:, 0:2, :, :])
            nc.scalar.dma_start(WDR[:, 2:8, :, :], wdr_d[:, 2:8, :, :])
            nc.scalar.dma_start(Wt[:], wt_d[:])
            nc.scalar.dma_start(MP[:], mp_d[:])
            nc.scalar.dma_start(Wext[:, 0 : 4 * DX], w0_d[:])
            u_sb = cw.tile([DX, NG, 8, GW], fp8)
            # u group 0 in quarters (one per DR matmul) so phase A streams
            # with DMA arrival; later groups in single DMAs.
            for h in range(4):
                nc.sync.dma_start(
                    u_sb[:, 0, 2 * h : 2 * h + 2, :],
                    u_d[0][:, h * 2 * GW : (h + 1) * 2 * GW],
                )
            for g in range(1, NG):
                nc.sync.dma_start(u_sb[:, g, :, :], u_d[g][:])

            xr = [
                cw.tile([DX, (L + 1) * GW], bf16, name=f"xr{g}") for g in range(NG)
            ]
            psCt = [psC.tile([DX, GW], f32, name=f"psCt{g}") for g in range(NG)]

            # PE warm-up while input DMAs are in flight: 8 small N=128
            # matmuls, then 4 N=512 zero-matmuls that pre-fill the psC state
            # banks (zeros + has_written set on every element) so phase B'
            # can ACT-copy its d=0 identity term instead of a matmul.
            scr = cw.tile([DX, GW], bf16)
            nc.gpsimd.memset(scr[:], 0)
            psw = psW.tile([DX, DX], f32)
            for _ in range(8):
                nc.tensor.matmul(psw[:], scr[:, 0:DX], scr[:, 0:DX], start=True, stop=True)
            for g in range(NG):
                nc.tensor.matmul(
                    psCt[g][:], scr[:, 0:DX], scr[:], start=True, stop=True
                )

            def ccopy(idx, dst, src):
                # alternate DVE / ACT for the per-step psum->bf16 copy
                if idx % 2 == 0:
                    nc.vector.tensor_copy(dst, src)
                else:
                    nc.scalar.copy(dst, src)

            for g in range(NG):
                # phase A: W for the 4 chunks of group g (fp8 DoubleRow,
                # each matmul contracts 2 pair-packed u blocks = 4 u steps)
                ps = psA.tile([DX, GW], f32)
                for m in range(4):
                    nc.tensor.matmul(
                        ps[:],
                        WDR[:, m, :, :],
                        u_sb[:, g, 2 * m : 2 * m + 2, :],
                        start=(m == 0),
                        stop=(m == 3),
                        perf_mode=DR,
                    )
                nc.scalar.mul(
                    Wext[:, (4 + g * GS) * DX : (4 + (g + 1) * GS) * DX],
                    ps[:],
                    1.0 / SA,
                )
                # phase B': d=0 identity term is an ACT copy into the
                # pre-filled (zeros, has_written set) psC bank; d=1..3
                # matmuls accumulate on top (+ chain term for g>0).
                sc0 = (4 * g + 3) * DX
                nc.scalar.copy(psCt[g][:], Wext[:, sc0 : sc0 + GW])
                for d in range(1, 4):
                    sc = (4 * g + 3 - d) * DX
                    nc.tensor.matmul(
                        psCt[g][:],
                        MP[:, d * DX : (d + 1) * DX],
                        Wext[:, sc : sc + GW],
                        start=False,
                        stop=False,
                        skip_group_check=True,
                    )
                if g > 0:
                    nc.tensor.matmul(
                        psCt[g][:],
                        MP[:, 4 * DX : 5 * DX],
                        xr[g - 1][:, 0:GW],
                        start=False,
                        stop=False,
                    )
                ccopy(g, xr[g][:, 0:GW], psCt[g][:])

            # phase C: psum IS the state; one copy per step per group
            ci = 0
            for k in range(1, L + 1):
                j = k - 1
                par = j & 1
                i = j >> 1
                for g in range(NG):
                    nc.tensor.matmul(
                        psCt[g][:],
                        Wt[:, par * DX : (par + 1) * DX],
                        u_sb[:, g, i, :],
                        start=False,
                        stop=False,
                    )
                    nc.tensor.matmul(
                        psCt[g][:],
                        Wt[:, 2 * DX : 3 * DX],
                        xr[g][:, (k - 1) * GW : k * GW],
                        start=False,
                        stop=(k == L),
                    )
                    ccopy(ci, xr[g][:, k * GW : (k + 1) * GW], psCt[g][:])
                    ci += 1
                    # 2-step output slabs on sync for k<=12
                    if k % 2 == 0 and k <= 12:
                        m = k // 2 - 1
                        nc.sync.dma_start(
                            y_d[g][m],
                            xr[g][:, (2 * m + 1) * GW : (2 * m + 3) * GW],
                        )
                # Tail descs (k=14 slab, k=15/16 singles) go AFTER all four
                # copies of that step, split across both queues, so a desc
                # waiting on one group's copy never blocks another group's
                # copy on the same engine queue.
                if k == 14:
                    for g in range(NG):
                        eng = [nc.sync, nc.sync, nc.scalar, nc.scalar][g]
                        eng.dma_start(y_d[g][6], xr[g][:, 13 * GW : 15 * GW])
                elif k == 15:
                    for g in range(NG):
                        eng = [nc.scalar, nc.scalar, nc.sync, nc.sync][g]
                        eng.dma_start(
                            y_d[g][7][:, 0:GW], xr[g][:, 15 * GW : 16 * GW]
                        )
                elif k == 16:
                    for g in range(NG):
                        eng = [nc.sync, nc.sync, nc.scalar, nc.scalar][g]
                        eng.dma_start(
                            y_d[g][7][:, GW : 2 * GW], xr[g][:, 16 * GW : 17 * GW]
                        )
    nc.compile()
    return nc


def _get_nc():
    if "nc" not in _CACHE:
        _CACHE["nc"] = _build()
    return _CACHE["nc"]


def _host_mats(A, Bmat):
    M64 = np.eye(DX, dtype=np.float64) + DT * A.astype(np.float64)
    B264 = DT * Bmat.astype(np.float64)
    Np = []
    Mp = np.eye(DX, dtype=np.float64)
    for p in range(L):
        Np.append((B264 @ Mp).astype(np.float32))
        Mp = Mp @ M64
    ML64 = Mp  # M^L
    WDR = np.zeros((DX, 8, 2, DX), dtype=np.float32)
    for m in range(4):
        for h in range(2):
            i = 2 * m + h
            WDR[0:DU, m, h, :] = Np[15 - 2 * i] * SA
            WDR[DU : 2 * DU, m, h, :] = Np[14 - 2 * i] * SA
    B2 = B264.astype(np.float32)
    Wt = np.zeros((DX, 3 * DX), dtype=np.float32)
    Wt[0:DU, 0:DX] = B2
    Wt[DU : 2 * DU, DX : 2 * DX] = B2
    Wt[:, 2 * DX : 3 * DX] = (DT * A.astype(np.float64)).astype(np.float32)
    MP = np.zeros((DX, 5 * DX), dtype=np.float32)
    Md = np.eye(DX, dtype=np.float64)
    for d in range(5):
        MP[:, d * DX : (d + 1) * DX] = Md.astype(np.float32)
        Md = Md @ ML64
    return (
        WDR.astype(ml_dtypes.float8_e4m3),
        Wt.astype(ml_dtypes.bfloat16),
        MP.astype(ml_dtypes.bfloat16),
    )


def _prep_inputs(initial_state, u_traj, A, Bmat):
    WDR, Wt, MP = _host_mats(A, Bmat)
    in_maps = []
    for c in range(NCORES):
        rc = slice(c * BPC, (c + 1) * BPC)
        w0 = np.zeros((DX, 4 * DX), dtype=np.float32)
        w0[:, 3 * DX :] = initial_state[rc].T
        uc = u_traj[rc]  # [b, t, du]; t = (4g+q)*16 + 2i+par
        ut = uc.reshape(BPC, NG, GS, 8, 2, DU)  # b, g, q, i, par, du
        ut = ut.transpose(1, 4, 5, 3, 2, 0)  # g, par, du, i, q, b
        uT = (
            np.ascontiguousarray(ut)
            .reshape(NG, DX, 8 * GW)
            .astype(ml_dtypes.float8_e4m3)
        )
        in_maps.append(
            {
                "WDR": WDR,
                "WT": Wt,
                "MP": MP,
                "W0T": w0.astype(ml_dtypes.bfloat16),
                "uT": uT,
            }
        )
    return in_maps


def _assemble(results, initial_state):
    out = np.empty((BATCH, T + 1, DX), dtype=np.float32)
    out[:, 0, :] = initial_state
    for c in range(NCORES):
        rc = slice(c * BPC, (c + 1) * BPC)
        yT = results[c]["yT"]  # [g, m, dx, kin*q*b] bf16
        y = np.asarray(yT).reshape(NG, 8, DX, 2, GS, BPC)  # g, m, dx, kin, q, b
        y = y.transpose(5, 0, 4, 1, 3, 2)  # b, g, q, m, kin, dx
        out[rc, 1:, :] = y.reshape(BPC, T, DX).astype(np.float32)
    return out


def run(initial_state, u_traj, A, Bmat, trace=False, **trace_kwargs):
    from concourse.bass_utils import run_bass_kernel_spmd

    nc = _get_nc()
    in_maps = _prep_inputs(initial_state, u_traj, A, Bmat)
    res = run_bass_kernel_spmd(
        nc, in_maps, list(range(NCORES)), trace=trace, **trace_kwargs
    )
    out = _assemble(res.results, initial_state)
    return out, res


def kernel(initial_state, u_traj, A, Bmat):
    out, _ = run(initial_state, u_traj, A, Bmat)
    return out


# revision 13
# speedup vs baseline: 1.1190x; 1.1190x over previous
"""Trainium2 kernel for nn_LinearDynamics: chunked two-level scan, 8-core data parallel.

v7: all-bf16 matmul path (fp8 u), restructured front-end and tail.
    The pre-phase-C critical path is PE-bound (warmup + phase A + phase B),
    so v7 cuts PE work and idle there:
    - warmup trimmed to 16 N=128 matmuls + 4 N=512 zero-matmuls, all inside
      the DMA-wait window [7.1us, 8.8us]; phase A starts as soon as the
      first u/Wt slices land and finishes warming HAM on real work.
    - the 4 N=512 zero-matmuls pre-fill the psC state banks (has_written
      set), letting phase B' ACT-copy its d=0 identity term instead of
      spending 4 PE matmuls.
    - u group 0 in quarters and group 3 in halves so phase A g0 streams
      with DMA arrival and A-g3's post-landing serial tail shrinks.
    - tail output DMAs (k=14 slab, k=15/16 singles) split across both HWDGE
      queues and issued after the step's copies.

Per core (128 batch rows, state transposed xT [d_x=128, b=128]):
  x_{t+1} = x_t + (x_t @ dtA + u_t @ B2),  dtA = dt*A, B2 = dt*B, M = I + dtA

Chunks: S=16 chunks of L=16 steps, grouped NG=4 x GS=4 (GW=512 cols).
Host precomputes (float64, cast bf16):
  Wt slot i (i<8): rows 0:64 = N_{15-2i}, rows 64:128 = N_{14-2i}, N_p = B2@M^p
  Wt slot 8/9: B2 zero-padded to rows 0:64 / rows 64:128; slot 10: dtA
  MP_d = M^(d*L)  d=0..4

Phase A: W_s = sum_j u_{sL+j} @ N_{15-j}; u pair-packed on partitions,
  8 matmuls/group into PSUM; ACT copies psum -> Wext (bf16).
Phase B': boundary X-block(g) into psC[g]: d=0 term ACT-copied into the
  pre-filled psC bank, 3 windowed W-term matmuls accumulate on top (+ for
  g>0 one chain matmul X-block(g-1) @ M^{4L}). Group accumulation left OPEN.
Phase C: PE keeps accumulating into the same psum bank:
    psC[g] += u_j @ B2pad + xr_{k-1} @ dtA     (psum IS the f32 state)
  one copy per step (DVE/ACT alternate) -> bf16 xr tile = next matmul
  input AND DMA output.

Host-sim rel err ~3.8e-3.
"""

import ml_dtypes
import numpy as np

DT = 0.1
BATCH, T, DX, DU = 1024, 256, 128, 64
NCORES = 8
BPC = BATCH // NCORES  # 128
S, L = 16, 16
NG, GS = 4, 4
GW = GS * BPC  # 512

_CACHE = {}


def _build(debug=False):
    import concourse.mybir as mybir
    import concourse.tile as tile
    from concourse import bacc

    f32 = mybir.dt.float32
    bf16 = mybir.dt.bfloat16
    fp8 = mybir.dt.float8e4

    nc = bacc.Bacc("TRN2", target_bir_lowering=False, debug=debug)
    wt_d = nc.declare_dram_parameter("WT", [DX, 11 * DX], bf16, isOutput=False)
    mp_d = nc.declare_dram_parameter("MP", [DX, 5 * DX], bf16, isOutput=False)
    w0_d = nc.declare_dram_parameter("W0T", [DX, 4 * DX], bf16, isOutput=False)
    u_d = nc.declare_dram_parameter("uT", [NG, DX, 8 * GW], fp8, isOutput=False)
    y_d = nc.declare_dram_parameter("yT", [NG, 8, DX, 2 * GW], bf16, isOutput=True)

    with tile.TileContext(nc) as tc:
        with (
            tc.tile_pool(name="cw", bufs=1) as cw,
            tc.tile_pool(name="psA", bufs=2, space="PSUM") as psA,
            tc.tile_pool(name="psW", bufs=1, space="PSUM") as psW,
            tc.tile_pool(name="psC", bufs=1, space="PSUM") as psC,
        ):
            # Input DMAs, priority-ordered: weights on the ACT queue, u on
            # the SYNC queue. First slices sized so phase A can start ~8.8us
            # and stream with DMA arrival (~0.3 MB/us aggregate input bw).
            Wt = cw.tile([DX, 11 * DX], bf16)
            nc.scalar.dma_start(Wt[:, 0 : 2 * DX], wt_d[:, 0 : 2 * DX])
            nc.scalar.dma_start(Wt[:, 2 * DX : 4 * DX], wt_d[:, 2 * DX : 4 * DX])
            nc.scalar.dma_start(Wt[:, 4 * DX :], wt_d[:, 4 * DX :])
            u_sb = cw.tile([DX, NG * 8 * GW], fp8)

            def udma(g, b0, b1):  # load u pair-blocks [b0,b1) of group g
                c0, c1 = (g * 8 + b0) * GW, (g * 8 + b1) * GW
                nc.sync.dma_start(u_sb[:, c0:c1], u_d[g][:, b0 * GW : b1 * GW])

            for h in range(4):  # group 0 in quarters
                udma(0, 2 * h, 2 * h + 2)
            MP = cw.tile([DX, 5 * DX], bf16)
            nc.scalar.dma_start(MP[:], mp_d[:])
            Wext = cw.tile([DX, (4 + S) * DX], bf16)
            nc.scalar.dma_start(Wext[:, 0 : 4 * DX], w0_d[:])
            udma(1, 0, 8)
            udma(2, 0, 8)
            udma(3, 0, 4)  # group 3 in halves: shrink A-g3's serial tail
            udma(3, 4, 8)

            xr = [
                cw.tile([DX, (L + 1) * GW], bf16, name=f"xr{g}") for g in range(NG)
            ]
            psCt = [psC.tile([DX, GW], f32, name=f"psCt{g}") for g in range(NG)]

            # PE warm-up inside the DMA-wait window: 16 N=128 matmuls, then
            # 4 N=512 zero-matmuls that pre-fill the psC state banks (zeros,
            # has_written set) for phase B's d=0 ACT copy.
            scr = cw.tile([DX, GW], bf16)
            nc.gpsimd.memset(scr[:], 0)
            psw = psW.tile([DX, DX], f32)
            for _ in range(16):
                nc.tensor.matmul(
                    psw[:], scr[:, 0:DX], scr[:, 0:DX], start=True, stop=True
                )
            for g in range(NG):
                nc.tensor.matmul(
                    psCt[g][:], scr[:, 0:DX], scr[:], start=True, stop=True
                )

            def ccopy(idx, dst, src):
                # alternate DVE / ACT for the per-step psum->bf16 copy
                if idx % 2 == 0:
                    nc.vector.tensor_copy(dst, src)
                else:
                    nc.scalar.copy(dst, src)

            for g in range(NG):
                # phase A: W for the 4 chunks of group g
                ps = psA.tile([DX, GW], f32)
                for i in range(8):
                    nc.tensor.matmul(
                        ps[:],
                        Wt[:, i * DX : (i + 1) * DX],
                        u_sb[:, g * 8 * GW + i * GW : g * 8 * GW + (i + 1) * GW],
                        start=(i == 0),
                        stop=(i == 7),
                    )
                nc.scalar.copy(
                    Wext[:, (4 + g * GS) * DX : (4 + (g + 1) * GS) * DX], ps[:]
                )
                # phase B': d=0 identity term is an ACT copy into the
                # pre-filled psC bank; d=1..3 matmuls accumulate on top
                # (+ chain term for g>0); group accumulation left OPEN.
                sc0 = (4 * g + 3) * DX
                nc.scalar.copy(psCt[g][:], Wext[:, sc0 : sc0 + GW])
                for d in range(1, 4):
                    sc = (4 * g + 3 - d) * DX
                    nc.tensor.matmul(
                        psCt[g][:],
                        MP[:, d * DX : (d + 1) * DX],
                        Wext[:, sc : sc + GW],
                        start=False,
                        stop=False,
                        skip_group_check=True,
                    )
                if g > 0:
                    nc.tensor.matmul(
                        psCt[g][:],
                        MP[:, 4 * DX : 5 * DX],
                        xr[g - 1][:, 0:GW],
                        start=False,
                        stop=False,
                    )
                ccopy(g, xr[g][:, 0:GW], psCt[g][:])

            # phase C: psum IS the state; one copy per step per group
            ci = 0
            for k in range(1, L + 1):
                j = k - 1
                par = j & 1
                i = j >> 1
                for g in range(NG):
                    nc.tensor.matmul(
                        psCt[g][:],
                        Wt[:, (8 + par) * DX : (9 + par) * DX],
                        u_sb[:, g * 8 * GW + i * GW : g * 8 * GW + (i + 1) * GW],
                        start=False,
                        stop=False,
                    )
                    nc.tensor.matmul(
                        psCt[g][:],
                        Wt[:, 10 * DX : 11 * DX],
                        xr[g][:, (k - 1) * GW : k * GW],
                        start=False,
                        stop=(k == L),
                    )
                    ccopy(ci, xr[g][:, k * GW : (k + 1) * GW], psCt[g][:])
                    ci += 1
                    # 2-step output slabs on sync for k<=12
                    if k % 2 == 0 and k <= 12:
                        m = k // 2 - 1
                        nc.sync.dma_start(
                            y_d[g][m],
                            xr[g][:, (2 * m + 1) * GW : (2 * m + 3) * GW],
                        )
                # Tail descs (k=14 slab, k=15/16 singles) go AFTER all four
                # copies of that step, split across both queues, so a desc
                # waiting on one group's copy never blocks another group's
                # copy on the same engine queue.
                if k == 14:
                    for g in range(NG):
                        eng = [nc.sync, nc.sync, nc.scalar, nc.scalar][g]
                        eng.dma_start(y_d[g][6], xr[g][:, 13 * GW : 15 * GW])
                elif k == 15:
                    for g in range(NG):
                        eng = [nc.scalar, nc.scalar, nc.sync, nc.sync][g]
                        eng.dma_start(
                            y_d[g][7][:, 0:GW], xr[g][:, 15 * GW : 16 * GW]
                        )
                elif k == 16:
                    for g in range(NG):
                        eng = [nc.sync, nc.sync, nc.scalar, nc.scalar][g]
                        eng.dma_start(
                            y_d[g][7][:, GW : 2 * GW], xr[g][:, 16 * GW : 17 * GW]
                        )
    nc.compile()
    return nc


def _get_nc():
    if "nc" not in _CACHE:
        _CACHE["nc"] = _build()
    return _CACHE["nc"]


def _host_mats(A, Bmat):
    M64 = np.eye(DX, dtype=np.float64) + DT * A.astype(np.float64)
    B264 = DT * Bmat.astype(np.float64)
    Np = []
    Mp = np.eye(DX, dtype=np.float64)
    for p in range(L):
        Np.append((B264 @ Mp).astype(np.float32))
        Mp = Mp @ M64
    ML64 = Mp  # M^L
    Wt = np.zeros((DX, 11 * DX), dtype=np.float32)
    for i in range(8):
        Wt[0:DU, i * DX : (i + 1) * DX] = Np[15 - 2 * i]
        Wt[DU : 2 * DU, i * DX : (i + 1) * DX] = Np[14 - 2 * i]
    B2 = B264.astype(np.float32)
    Wt[0:DU, 8 * DX : 9 * DX] = B2
    Wt[DU : 2 * DU, 9 * DX : 10 * DX] = B2
    Wt[:, 10 * DX : 11 * DX] = (DT * A.astype(np.float64)).astype(np.float32)
    MP = np.zeros((DX, 5 * DX), dtype=np.float32)
    Md = np.eye(DX, dtype=np.float64)
    for d in range(5):
        MP[:, d * DX : (d + 1) * DX] = Md.astype(np.float32)
        Md = Md @ ML64
    return Wt.astype(ml_dtypes.bfloat16), MP.astype(ml_dtypes.bfloat16)


def _prep_inputs(initial_state, u_traj, A, Bmat):
    Wt, MP = _host_mats(A, Bmat)
    in_maps = []
    for c in range(NCORES):
        rc = slice(c * BPC, (c + 1) * BPC)
        w0 = np.zeros((DX, 4 * DX), dtype=np.float32)
        w0[:, 3 * DX :] = initial_state[rc].T
        uc = u_traj[rc]  # [b, t, du]; t = (4g+q)*16 + 2i+par
        ut = uc.reshape(BPC, NG, GS, 8, 2, DU)  # b, g, q, i, par, du
        ut = ut.transpose(1, 4, 5, 3, 2, 0)  # g, par, du, i, q, b
        uT = (
            np.ascontiguousarray(ut)
            .reshape(NG, DX, 8 * GW)
            .astype(ml_dtypes.float8_e4m3)
        )
        in_maps.append(
            {
                "WT": Wt,
                "MP": MP,
                "W0T": w0.astype(ml_dtypes.bfloat16),
                "uT": uT,
            }
        )
    return in_maps


def _assemble(results, initial_state):
    out = np.empty((BATCH, T + 1, DX), dtype=np.float32)
    out[:, 0, :] = initial_state
    for c in range(NCORES):
        rc = slice(c * BPC, (c + 1) * BPC)
        yT = results[c]["yT"]  # [g, m, dx, kin*q*b] bf16
        y = np.asarray(yT).reshape(NG, 8, DX, 2, GS, BPC)  # g, m, dx, kin, q, b
        y = y.transpose(5, 0, 4, 1, 3, 2)  # b, g, q, m, kin, dx
        out[rc, 1:, :] = y.reshape(BPC, T, DX).astype(np.float32)
    return out


def run(initial_state, u_traj, A, Bmat, trace=False, **trace_kwargs):
    from concourse.bass_utils import run_bass_kernel_spmd

    nc = _get_nc()
    in_maps = _prep_inputs(initial_state, u_traj, A, Bmat)
    res = run_bass_kernel_spmd(
        nc, in_maps, list(range(NCORES)), trace=trace, **trace_kwargs
    )
    out = _assemble(res.results, initial_state)
    return out, res


def kernel(initial_state, u_traj, A, Bmat):
    out, _ = run(initial_state, u_traj, A, Bmat)
    return out
